# revision 2
# baseline (speedup 1.0000x reference)
"""CapsuleLayer forward on 8 Trainium2 NeuronCores.

The reference collapses algebraically: routing runs exactly one iteration
with uniform coefficients c = 1/R, so

    s[b, (n,o)] = (1/R) * sum_{r,i} x[b,r,i] * W[n,r,i,o]
                = (1/R) * (x_flat @ W_flat)[b, (n,o)]
    v = squash(s) over o

i.e. one [256, 9216] @ [9216, 160] matmul plus a tiny squash. u_hat is
never materialized.

Sharding: contraction-dim (K = R*CIN) sharding across the 8 cores. Each
core loads 1/8 of x^T and 1/8 of W (1.9 MB total), accumulates its partial
s in PSUM, then a ReduceScatter(add) over the 8 cores hands each core a
32-batch slice of the full sum; squash runs on-device; the host
concatenates the 8 slices.
"""

import numpy as np
from contextlib import ExitStack

import concourse.bass as bass
import concourse.tile as tile
from concourse import bacc, mybir
from concourse.bass_utils import run_bass_kernel_spmd

N_CAPS, R, CIN, COUT = 10, 1152, 8, 16
B = 256
NCORES = 8
K = R * CIN            # 9216 contraction length
KSH = K // NCORES      # 1152 contraction slice per core
NO = N_CAPS * COUT     # 160 output columns
P = 128
KT = KSH // P          # 9 k-tiles per core
BSH = B // NCORES      # 32 batch rows per core after ReduceScatter

F32 = mybir.dt.float32

_built = None


def _build_nc():
    nc = bacc.Bacc(
        "TRN2", target_bir_lowering=False, debug=False, num_devices=NCORES
    )
    xt = nc.dram_tensor("xt", [KSH, B], F32, kind="ExternalInput").ap()
    wk = nc.dram_tensor("wk", [KSH, NO], F32, kind="ExternalInput").ap()
    out = nc.dram_tensor("out", [BSH, NO], F32, kind="ExternalOutput").ap()

    with tile.TileContext(nc) as tc, ExitStack() as ctx:
        xp = ctx.enter_context(tc.tile_pool(name="xp", bufs=3))
        wp = ctx.enter_context(tc.tile_pool(name="wp", bufs=3))
        pp = ctx.enter_context(tc.tile_pool(name="pp", bufs=1, space="PSUM"))
        sb = ctx.enter_context(tc.tile_pool(name="sb", bufs=1))
        dr = ctx.enter_context(tc.tile_pool(name="dr", bufs=1, space="DRAM"))

        # Partial s for this core's k-slice: [256, 160] split across two
        # PSUM tiles (stationary operand is limited to 128 columns).
        ps0 = pp.tile([P, NO], F32, tag="ps0")
        ps1 = pp.tile([P, NO], F32, tag="ps1")
        for k in range(KT):
            xt_t = xp.tile([P, B], F32)
            nc.sync.dma_start(xt_t[:], xt[k * P:(k + 1) * P, :])
            wk_t = wp.tile([P, NO], F32)
            nc.sync.dma_start(wk_t[:], wk[k * P:(k + 1) * P, :])
            nc.tensor.matmul(
                ps0[:], xt_t[:, 0:P], wk_t[:],
                start=(k == 0), stop=(k == KT - 1),
            )
            nc.tensor.matmul(
                ps1[:], xt_t[:, P:2 * P], wk_t[:],
                start=(k == 0), stop=(k == KT - 1),
            )

        part0 = sb.tile([P, NO], F32, tag="part0")
        part1 = sb.tile([P, NO], F32, tag="part1")
        nc.vector.tensor_copy(part0[:], ps0[:])
        nc.vector.tensor_copy(part1[:], ps1[:])

        rs_in = dr.tile([B, NO], F32, tag="rsin")
        rs_out = dr.tile([BSH, NO], F32, tag="rsout")
        nc.sync.dma_start(rs_in[0:P, :], part0[:])
        nc.sync.dma_start(rs_in[P:2 * P, :], part1[:])
        nc.gpsimd.collective_compute(
            "ReduceScatter",
            mybir.AluOpType.add,
            replica_groups=[list(range(NCORES))],
            ins=[rs_in.opt()],
            outs=[rs_out.opt()],
        )

        # t = full raw sum for this core's 32 batches. With s = t/R and
        # sq = sum_o s^2 = ssq/R^2, squash reduces to
        #   v = t * sqrt(ssq) / (R^2 + ssq)
        t = sb.tile([BSH, NO], F32, tag="t")
        nc.sync.dma_start(t[:], rs_out[:])

        sq = sb.tile([BSH, NO], F32, tag="sq")
        nc.vector.tensor_mul(sq[:], t[:], t[:])
        ssq = sb.tile([BSH, N_CAPS], F32, tag="ssq")
        nc.vector.tensor_reduce(
            ssq[:],
            sq[:].rearrange("p (n o) -> p n o", o=COUT),
            axis=mybir.AxisListType.X,
            op=mybir.AluOpType.add,
        )
        rt = sb.tile([BSH, N_CAPS], F32, tag="rt")
        nc.scalar.sqrt(rt[:], ssq[:])
        den = sb.tile([BSH, N_CAPS], F32, tag="den")
        nc.vector.tensor_scalar_add(den[:], ssq[:], float(R * R))
        rec = sb.tile([BSH, N_CAPS], F32, tag="rec")
        nc.vector.reciprocal(rec[:], den[:])
        sc = sb.tile([BSH, N_CAPS], F32, tag="sc")
        nc.vector.tensor_mul(sc[:], rt[:], rec[:])

        v = sb.tile([BSH, NO], F32, tag="v")
        nc.vector.tensor_mul(
            v[:].rearrange("p (n o) -> p n o", o=COUT),
            t[:].rearrange("p (n o) -> p n o", o=COUT),
            sc[:].unsqueeze(2).broadcast_to([BSH, N_CAPS, COUT]),
        )
        nc.sync.dma_start(out[:], v[:])

    nc.compile()
    return nc


def _get_nc():
    global _built
    if _built is None:
        _built = _build_nc()
    return _built


def _make_in_maps(x, W):
    x = np.asarray(x, dtype=np.float32)
    W = np.asarray(W, dtype=np.float32)
    # x^T: [K, B]; W to [K, NO] with k = r*CIN + i matching x's flattening.
    xt_full = np.ascontiguousarray(x.reshape(B, K).T)
    wk_full = np.ascontiguousarray(W.transpose(1, 2, 0, 3).reshape(K, NO))
    return [
        {
            "xt": xt_full[c * KSH:(c + 1) * KSH],
            "wk": wk_full[c * KSH:(c + 1) * KSH],
        }
        for c in range(NCORES)
    ]


def _assemble(results):
    v = np.concatenate([results[c]["out"] for c in range(NCORES)], axis=0)
    v = v.reshape(B, N_CAPS, COUT).transpose(1, 0, 2)
    return np.ascontiguousarray(v[:, :, None, None, :]).astype(np.float32)


def _run(x, W, **spmd_kwargs):
    nc = _get_nc()
    in_maps = _make_in_maps(x, W)
    return run_bass_kernel_spmd(nc, in_maps, list(range(NCORES)), **spmd_kwargs)


def kernel(x, W):
    res = _run(x, W)
    return _assemble(res.results)


# revision 4
# speedup vs baseline: 2.8293x; 2.8293x over previous
"""CapsuleLayer forward on 8 Trainium2 NeuronCores.

The reference collapses algebraically: routing runs exactly one iteration
with uniform coefficients c = 1/R, so

    s[b, (n,o)] = (1/R) * sum_{r,i} x[b,r,i] * W[n,r,i,o]
                = (1/R) * (x_flat @ W_flat)[b, (n,o)]
    v = squash(s) over o

i.e. one [256, 9216] @ [9216, 160] matmul plus a tiny squash on 40960
elements. u_hat ([10,256,1152,16], 189 MB) is never materialized.

Sharding: contraction-dim (K = R*CIN) sharding across the 8 cores — each
core reads only 1/8 of x^T and 1/8 of W (1.9 MB vs 7 MB/core for batch
sharding) and the PE does 4x fewer matmul instructions (full 128-wide
stationary operand). Each core emits its raw partial product (s^T layout,
[160, 256]); the host sums the 8 partials and applies the tiny squash
while unsharding. (An on-device ReduceScatter was measured at ~46 us —
a 32 us launch-skew barrier plus 14 us transfer — so cross-core
reduction on device is strictly worse.)

Matmuls run as float32r (fp32 bits, replicated PE mode): with a 256-wide
moving operand this runs at 1 cycle/row vs 4 for plain fp32.
"""

import numpy as np
from contextlib import ExitStack

import concourse.bass as bass
import concourse.tile as tile
from concourse import bacc, mybir
from concourse.bass_utils import run_bass_kernel_spmd

N_CAPS, R, CIN, COUT = 10, 1152, 8, 16
B = 256
NCORES = 8
K = R * CIN            # 9216 contraction length
KSH = K // NCORES      # 1152 contraction slice per core
NO = N_CAPS * COUT     # 160 output rows (s^T layout)
P = 128
KT = KSH // P          # 9 k-tiles per core

F32 = mybir.dt.float32
F32R = mybir.dt.float32r

_built = None


def _build_nc():
    nc = bacc.Bacc(
        "TRN2", target_bir_lowering=False, debug=False, num_devices=NCORES
    )
    xt = nc.dram_tensor("xt", [KSH, B], F32R, kind="ExternalInput").ap()
    wk = nc.dram_tensor("wk", [KSH, NO], F32R, kind="ExternalInput").ap()
    out = nc.dram_tensor("out", [NO, B], F32, kind="ExternalOutput").ap()

    with tile.TileContext(nc) as tc, ExitStack() as ctx:
        xp = ctx.enter_context(tc.tile_pool(name="xp", bufs=3))
        wp = ctx.enter_context(tc.tile_pool(name="wp", bufs=3))
        pp = ctx.enter_context(tc.tile_pool(name="pp", bufs=1, space="PSUM"))

        # s^T partial: [160, 256] across two PSUM tiles (stationary W
        # tile is limited to 128 columns).
        psA = pp.tile([P, B], F32, tag="psA")
        psB = pp.tile([NO - P, B], F32, tag="psB")
        for k in range(KT):
            xt_t = xp.tile([P, B], F32R)
            nc.sync.dma_start(xt_t[:], xt[k * P:(k + 1) * P, :])
            wk_t = wp.tile([P, NO], F32R)
            nc.sync.dma_start(wk_t[:], wk[k * P:(k + 1) * P, :])
            nc.tensor.matmul(
                psA[:], wk_t[:, 0:P], xt_t[:],
                start=(k == 0), stop=(k == KT - 1),
            )
            nc.tensor.matmul(
                psB[:], wk_t[:, P:NO], xt_t[:],
                start=(k == 0), stop=(k == KT - 1),
            )
        sb = ctx.enter_context(tc.tile_pool(name="sb", bufs=1))
        sA = sb.tile([P, B], F32, tag="sA")
        sB = sb.tile([NO - P, B], F32, tag="sB")
        nc.vector.tensor_copy(sA[:], psA[:])
        nc.vector.tensor_copy(sB[:], psB[:])
        nc.sync.dma_start(out[0:P, :], sA[:])
        nc.sync.dma_start(out[P:NO, :], sB[:])

    nc.compile()
    return nc


def _get_nc():
    global _built
    if _built is None:
        _built = _build_nc()
    return _built


def _make_in_maps(x, W):
    x = np.asarray(x, dtype=np.float32)
    W = np.asarray(W, dtype=np.float32)
    # x^T: [K, B]; W to [K, NO] with k = r*CIN + i matching x's flattening.
    xt_full = np.ascontiguousarray(x.reshape(B, K).T)
    wk_full = np.ascontiguousarray(W.transpose(1, 2, 0, 3).reshape(K, NO))
    return [
        {
            "xt": xt_full[c * KSH:(c + 1) * KSH],
            "wk": wk_full[c * KSH:(c + 1) * KSH],
        }
        for c in range(NCORES)
    ]


def _assemble(results):
    # Sum the 8 K-slice partials (the "all-reduce" leg of unsharding),
    # then apply squash: with t = raw sum (s = t/R, ssq = sum_o t^2),
    #   v = t * sqrt(ssq) / (R^2 + ssq)
    t = np.zeros((NO, B), dtype=np.float32)
    for c in range(NCORES):
        t += results[c]["out"]
    t = t.T.reshape(B, N_CAPS, COUT).astype(np.float64)
    ssq = np.sum(t * t, axis=-1, keepdims=True)
    v = t * np.sqrt(ssq) / (R * R + ssq)
    return np.ascontiguousarray(
        v.transpose(1, 0, 2)[:, :, None, None, :]
    ).astype(np.float32)


def _run(x, W, **spmd_kwargs):
    nc = _get_nc()
    in_maps = _make_in_maps(x, W)
    return run_bass_kernel_spmd(nc, in_maps, list(range(NCORES)), **spmd_kwargs)


def kernel(x, W):
    res = _run(x, W)
    return _assemble(res.results)


# revision 6
# speedup vs baseline: 3.5370x; 1.2502x over previous
"""CapsuleLayer forward on 8 Trainium2 NeuronCores.

The reference collapses algebraically: routing runs exactly one iteration
with uniform coefficients c = 1/R, so

    s[b, (n,o)] = (1/R) * sum_{r,i} x[b,r,i] * W[n,r,i,o]
                = (1/R) * (x_flat @ W_flat)[b, (n,o)]
    v = squash(s) over o

i.e. one [256, 9216] @ [9216, 160] matmul plus a tiny squash on 40960
elements. u_hat ([10,256,1152,16], 189 MB) is never materialized.

Sharding: contraction-dim (K = R*CIN) sharding across the 8 cores — each
core reads only 1/8 of x^T and 1/8 of W (1.9 MB vs 7 MB/core for batch
sharding) and the PE does 4x fewer matmul instructions (full 128-wide
stationary operand). Each core emits its raw partial product (s^T layout,
[160, 256]); the host sums the 8 partials and applies the tiny squash
while unsharding. (An on-device ReduceScatter was measured at ~46 us —
a 32 us launch-skew barrier plus 14 us transfer — so cross-core
reduction on device is strictly worse.)

Matmuls run as float32r (fp32 bits, replicated PE mode): with a 256-wide
moving operand this runs at 1 cycle/row vs 4 for plain fp32.
"""

import numpy as np
from contextlib import ExitStack

import concourse.bass as bass
import concourse.tile as tile
from concourse import bacc, mybir
from concourse.bass_utils import run_bass_kernel_spmd

N_CAPS, R, CIN, COUT = 10, 1152, 8, 16
B = 256
NCORES = 8
K = R * CIN            # 9216 contraction length
KSH = K // NCORES      # 1152 contraction slice per core
NO = N_CAPS * COUT     # 160 output rows (s^T layout)
P = 128
KT = KSH // P          # 9 k-tiles per core

F32 = mybir.dt.float32
F32R = mybir.dt.float32r

_built = None


CHUNKS = 3                 # pipelined input chunks
KPC = KT // CHUNKS         # k-tiles per chunk


def _build_nc():
    nc = bacc.Bacc(
        "TRN2", target_bir_lowering=False, debug=False, num_devices=NCORES
    )
    # Inputs are host-packed k-tile-major: row p holds, for each k-tile,
    # the p-th contraction row of that tile — so every DMA below is a
    # large per-partition-contiguous slab.
    xt = nc.dram_tensor("xt", [P, KT * B], F32R, kind="ExternalInput").ap()
    wk = nc.dram_tensor("wk", [P, KT * NO], F32R, kind="ExternalInput").ap()
    out = nc.dram_tensor("out", [NO, B], F32, kind="ExternalOutput").ap()

    with tile.TileContext(nc) as tc, ExitStack() as ctx:
        xp = ctx.enter_context(tc.tile_pool(name="xp", bufs=2))
        wp = ctx.enter_context(tc.tile_pool(name="wp", bufs=2))
        pp = ctx.enter_context(tc.tile_pool(name="pp", bufs=1, space="PSUM"))

        # s^T partial: [160, 256] across two PSUM tiles (stationary W
        # tile is limited to 128 columns).
        psA = pp.tile([P, B], F32, tag="psA")
        psB = pp.tile([NO - P, B], F32, tag="psB")
        for ch in range(CHUNKS):
            xt_t = xp.tile([P, KPC * B], F32R)
            nc.sync.dma_start(
                xt_t[:], xt[:, ch * KPC * B:(ch + 1) * KPC * B]
            )
            wk_t = wp.tile([P, KPC * NO], F32R)
            nc.scalar.dma_start(
                wk_t[:], wk[:, ch * KPC * NO:(ch + 1) * KPC * NO]
            )
            for j in range(KPC):
                k = ch * KPC + j
                nc.tensor.matmul(
                    psA[:],
                    wk_t[:, j * NO:j * NO + P],
                    xt_t[:, j * B:(j + 1) * B],
                    start=(k == 0), stop=(k == KT - 1),
                )
                nc.tensor.matmul(
                    psB[:],
                    wk_t[:, j * NO + P:(j + 1) * NO],
                    xt_t[:, j * B:(j + 1) * B],
                    start=(k == 0), stop=(k == KT - 1),
                )
        sb = ctx.enter_context(tc.tile_pool(name="sb", bufs=1))
        sA = sb.tile([P, B], F32, tag="sA")
        sB = sb.tile([NO - P, B], F32, tag="sB")
        nc.vector.tensor_copy(sA[:], psA[:])
        nc.vector.tensor_copy(sB[:], psB[:])
        nc.sync.dma_start(out[0:P, :], sA[:])
        nc.sync.dma_start(out[P:NO, :], sB[:])

    nc.compile()
    return nc


def _get_nc():
    global _built
    if _built is None:
        _built = _build_nc()
    return _built


def _make_in_maps(x, W):
    x = np.asarray(x, dtype=np.float32)
    W = np.asarray(W, dtype=np.float32)
    # x^T: [K, B]; W to [K, NO] with k = r*CIN + i matching x's flattening.
    # Then pack k-tile-major per core: [NCORES, P, KT * cols] where row p
    # holds k-tile k's p-th contraction row at column block k.
    xt_full = x.reshape(B, K).T  # [K, B] view
    wk_full = W.transpose(1, 2, 0, 3).reshape(K, NO)
    xt_pack = np.ascontiguousarray(
        xt_full.reshape(NCORES, KT, P, B).transpose(0, 2, 1, 3)
    ).reshape(NCORES, P, KT * B)
    wk_pack = np.ascontiguousarray(
        wk_full.reshape(NCORES, KT, P, NO).transpose(0, 2, 1, 3)
    ).reshape(NCORES, P, KT * NO)
    return [
        {"xt": xt_pack[c], "wk": wk_pack[c]}
        for c in range(NCORES)
    ]


def _assemble(results):
    # Sum the 8 K-slice partials (the "all-reduce" leg of unsharding),
    # then apply squash: with t = raw sum (s = t/R, ssq = sum_o t^2),
    #   v = t * sqrt(ssq) / (R^2 + ssq)
    t = np.zeros((NO, B), dtype=np.float32)
    for c in range(NCORES):
        t += results[c]["out"]
    t = t.T.reshape(B, N_CAPS, COUT).astype(np.float64)
    ssq = np.sum(t * t, axis=-1, keepdims=True)
    v = t * np.sqrt(ssq) / (R * R + ssq)
    return np.ascontiguousarray(
        v.transpose(1, 0, 2)[:, :, None, None, :]
    ).astype(np.float32)


def _run(x, W, **spmd_kwargs):
    nc = _get_nc()
    in_maps = _make_in_maps(x, W)
    return run_bass_kernel_spmd(nc, in_maps, list(range(NCORES)), **spmd_kwargs)


def kernel(x, W):
    res = _run(x, W)
    return _assemble(res.results)
